# revision 1
# baseline (speedup 1.0000x reference)
"""Causal self-attention (B=4, T=2048, E=1024, H=16) on 8 trn2 NeuronCores.

Sharding: core c -> (batch b = c // 2, head-group hg = c % 2); each core owns
one batch element and 8 of the 16 heads (data parallel on B, tensor parallel
on heads).  No cross-core communication.

Per-core device program (SPMD, same NEFF on all 8 cores), interleaved per
512-token block tb: QKV projection for tb, then attention for query block
I = tb (causal -> only needs k/v from blocks <= tb):
  qT,kT [c,t]-layout (2 heads packed per 128-partition tile), bias on DVE
  v     [t,c]-layout with a ones column per head, bias via K=1 matmul
  attention (all matmuls in 64-row PE tiling mode, no mode switches):
    St[j,i] strip [A|B]: QK row-tile pair computes 2 heads concurrently
    Pt = exp(St/8) on ScalarE (one instr per head pair), causal mask via a
         width-trimmed gpsimd affine_select on diagonal tiles only
    Yt[d|sum, i]: PV row-tile pair (j split 64+64) -> 2 psum partials,
         summed on DVE; softmax denominators come out as row 64
    y = Yt[:64] * (1/Yt[64]); the broadcast of the reciprocal across
        partitions is a K=1 matmul (ones^T @ recip)
Output written as yT [c, t]; the host transposes and concatenates.
"""

import sys

sys.path.insert(0, "/opt/trn_rl_repo")

import numpy as np

N_CORES = 8
B, T, E = 4, 2048, 1024
H, D = 16, 64
C = E                 # q/k/v channel count (4th qkv chunk unused)
HPC = H // 2          # heads per core
CC = HPC * D          # per-core channels = 512
ES = E // 128         # 8 e-tiles (contraction)
TB = T // 512         # 4 t/i blocks of 512
NJ = T // 128         # 16 j-tiles of 128
PAIRS = HPC // 2      # 4 head pairs per core

_cache = {}


def _build_nc():
    import concourse.mybir as mybir
    import concourse.tile as tile
    from concourse import bacc

    f32 = mybir.dt.float32
    f32r = mybir.dt.float32r
    Act = mybir.ActivationFunctionType
    is_ge = mybir.AluOpType.is_ge

    nc = bacc.Bacc("TRN2", target_bir_lowering=False, debug=False)

    xT = nc.dram_tensor("xT", [E, T], f32r, kind="ExternalInput").ap()
    w_qk = nc.dram_tensor("w_qk", [E, 2 * CC], f32r, kind="ExternalInput").ap()
    w_v = nc.dram_tensor("w_v", [E, CC], f32r, kind="ExternalInput").ap()
    b_qk = nc.dram_tensor("b_qk", [128, 8], f32, kind="ExternalInput").ap()
    b_v = nc.dram_tensor("b_v", [1, CC], f32r, kind="ExternalInput").ap()
    ones_d = nc.dram_tensor("ones_d", [1, 128], f32r, kind="ExternalInput").ap()
    yT = nc.dram_tensor("yT", [CC, T], f32, kind="ExternalOutput").ap()

    with tile.TileContext(nc) as tc:
        with (
            tc.tile_pool(name="persist", bufs=1) as pp,
            tc.tile_pool(name="psum", bufs=1, space="PSUM") as psp,
            tc.tile_pool(name="xpool", bufs=2) as xp,
            tc.tile_pool(name="ptpool", bufs=4) as ptp,
            tc.tile_pool(name="opool", bufs=1) as op,
            tc.tile_pool(name="dpool", bufs=2, space="DRAM") as dp,
        ):
            # ---- persistent SBUF state ----
            qk_sb = [pp.tile([128, T], f32r, name=f"qk{ct}") for ct in range(8)]
            # v plus a ones column per head: [t-part, head, t-tile, 65]
            v1_sb = pp.tile([128, HPC, NJ, D + 1], f32r, name="v1")
            bqk_sb = pp.tile([128, 8], f32, name="bqk")
            bv_sb = pp.tile([1, CC], f32r, name="bv")
            ones_sb = pp.tile([1, 128], f32r, name="ones")
            wqk_t = []
            wv_t = []

            # input DMAs: x(tb0) first so the first matmul group can start,
            # then weights, then the small vectors
            xs_tb = {}

            def load_x(tb):
                tsl = slice(tb * 512, (tb + 1) * 512)
                xs = []
                for e in range(ES):
                    xe = xp.tile([128, 512], f32r, tag=f"x{e}",
                                 bufs=(2 if e < 3 else 1), name=f"x{e}_{tb}")
                    nc.sync.dma_start(out=xe, in_=xT[e * 128 : (e + 1) * 128, tsl])
                    xs.append(xe)
                xs_tb[tb] = xs

            # small constants first, then x/w interleaved per e-tile so the
            # first matmul accumulation group can finish as early as possible
            nc.sync.dma_start(out=bqk_sb, in_=b_qk)
            nc.sync.dma_start(out=bv_sb, in_=b_v)
            nc.sync.dma_start(out=ones_sb, in_=ones_d)
            tsl0 = slice(0, 512)
            xs0 = []
            for e in range(ES):
                xe = xp.tile([128, 512], f32r, tag=f"x{e}",
                             bufs=(2 if e < 3 else 1), name=f"x{e}_0")
                nc.sync.dma_start(out=xe, in_=xT[e * 128 : (e + 1) * 128, tsl0])
                xs0.append(xe)
                wv = pp.tile([128, CC], f32r, name=f"wv{e}")
                nc.sync.dma_start(out=wv, in_=w_v[e * 128 : (e + 1) * 128, :])
                wv_t.append(wv)
            xs_tb[0] = xs0
            for e in range(ES):
                wqk = pp.tile([128, 2 * CC], f32r, name=f"wqk{e}")
                nc.sync.dma_start(out=wqk, in_=w_qk[e * 128 : (e + 1) * 128, :])
                wqk_t.append(wqk)
            ones_bc = _bcast_ap(ones_d, 128)
            nc.sync.dma_start(out=v1_sb[:, :, :, D : D + 1], in_=ones_bc)

            def qkv_group_qk(tb, ct):
                tsl = slice(tb * 512, (tb + 1) * 512)
                xs = xs_tb[tb]
                ps = psp.tile([128, 512], f32, tag="st", bufs=2,
                              name=f"psqk{ct}_{tb}")
                for e in range(ES):
                    nc.tensor.matmul(
                        ps,
                        wqk_t[e][:, ct * 128 : (ct + 1) * 128],
                        xs[e],
                        start=(e == 0),
                        stop=(e == ES - 1),
                    )
                nc.scalar.activation(
                    qk_sb[ct][:, tsl], ps, Act.Identity,
                    bias=bqk_sb[:, ct : ct + 1], scale=1.0)

            def qkv_group_v(tb, k4):
                xs = xs_tb[tb]
                tt = tb * 4 + k4
                psv = psp.tile([128, 512], f32, tag="st", bufs=2,
                               name=f"psv{tt}")
                nc.tensor.matmul(
                    psv, ones_sb, bv_sb,
                    start=True, stop=False, skip_group_check=True,
                )
                for e in range(ES):
                    nc.tensor.matmul(
                        psv,
                        xs[e][:, k4 * 128 : (k4 + 1) * 128],
                        wv_t[e],
                        start=False,
                        stop=(e == ES - 1),
                        skip_group_check=True,
                    )
                nc.vector.tensor_copy(
                    v1_sb[:, :, tt, 0:D],
                    psv.rearrange("p (h d) -> p h d", d=D),
                )

            def attn_block(I, nxt=()):
                isl = slice(I * 512, (I + 1) * 512)
                nj = 4 * I + 4  # causal j-tiles for this i-block
                yts = {}
                pts = {}

                def alloc_yt(pr):
                    yts[pr] = [
                        psp.tile([D + 1, 512], f32, tag=f"yt{n}",
                                 name=f"yt{n}_{pr}_{I}")
                        for n in ("A0", "A1", "B0", "B1")
                    ]

                def qk_exp(pr, J):
                    qt = qk_sb[pr]
                    kt = qk_sb[4 + pr]
                    jsl = slice(J * 128, (J + 1) * 128)
                    st = psp.tile([128, 1024], f32, tag="st", bufs=2,
                                  name=f"st{pr}_{I}_{J}")
                    # QK row-tile pair: head A rows 0-63, head B 64-127
                    nc.tensor.matmul(
                        st[:, 0:512], kt[0:64, jsl], qt[0:64, isl],
                        tile_position=(0, 0),
                    )
                    nc.tensor.matmul(
                        st[:, 512:1024], kt[64:128, jsl], qt[64:128, isl],
                        tile_position=(64, 0),
                    )
                    pt = ptp.tile([128, 1024], f32r, tag="pt",
                                  name=f"pt{pr}_{I}_{J}")
                    nc.scalar.activation(pt, st, Act.Exp, scale=0.125)
                    r = J - 4 * I
                    if r >= 0:  # diagonal tile: causal mask, trimmed width
                        w = (r + 1) * 128
                        for off in (0, 512):
                            # keep where (512I + y) - (128J + x) >= 0
                            nc.gpsimd.affine_select(
                                out=pt[:, off : off + w],
                                in_=pt[:, off : off + w],
                                compare_op=is_ge,
                                fill=0.0,
                                base=-128 * r,
                                pattern=[[1, w]],
                                channel_multiplier=-1,
                            )
                    pts[(pr, J)] = pt

                def pv(pr, J):
                    pt = pts.pop((pr, J))
                    ytA0, ytA1, ytB0, ytB1 = yts[pr]
                    first, last = (J == 0), (J == nj - 1)
                    # PV row-tile pairs (j contraction split 64+64)
                    nc.tensor.matmul(
                        ytA0, v1_sb[0:64, 2 * pr, J, :], pt[0:64, 0:512],
                        tile_position=(0, 0),
                        start=first, stop=last, skip_group_check=True,
                    )
                    nc.tensor.matmul(
                        ytA1, v1_sb[64:128, 2 * pr, J, :], pt[64:128, 0:512],
                        tile_position=(64, 0),
                        start=first, stop=last, skip_group_check=True,
                    )
                    nc.tensor.matmul(
                        ytB0, v1_sb[0:64, 2 * pr + 1, J, :], pt[0:64, 512:1024],
                        tile_position=(0, 0),
                        start=first, stop=last, skip_group_check=True,
                    )
                    nc.tensor.matmul(
                        ytB1, v1_sb[64:128, 2 * pr + 1, J, :],
                        pt[64:128, 512:1024],
                        tile_position=(64, 0),
                        start=first, stop=last, skip_group_check=True,
                    )

                def out_stage(pr):
                    ytA0, ytA1, ytB0, ytB1 = yts.pop(pr)
                    # ---- normalize + emit [128 rows = 2 heads, 512] ----
                    # copy + single add per head frees the yt psum slots after
                    # only two DVE ops; row 64 of tmp holds the softmax sum
                    ystage = op.tile([128, 512], f32, tag="ystage", bufs=2,
                                     name=f"ys{pr}_{I}")
                    rec2 = op.tile([33, 512], f32, tag="rec2",
                                   name=f"rec2{pr}_{I}")
                    tmpA = op.tile([D + 1, 512], f32, tag="tmpA",
                                   name=f"tmpA{pr}_{I}")
                    sum1 = op.tile([1, 512], f32, tag="sum1",
                                   name=f"sum1{pr}_{I}")
                    sA = op.tile([D + 1, 512], f32, tag="sA", name=f"sA{pr}_{I}")
                    sB = op.tile([D + 1, 512], f32, tag="sB", name=f"sB{pr}_{I}")
                    recA, recB = rec2[0:1, :], rec2[32:33, :]
                    # all yt-psum reads first so the four banks free as
                    # early as possible (next pair's PV waits on them); the
                    # reciprocals run after.  head A combines into tmpA
                    # (base 0 throughout); head B combines straight into
                    # ystage rows 64-127 so the final in-place mul keeps
                    # matching base partitions.
                    nc.vector.tensor_copy(sA, ytA1)
                    nc.vector.tensor_copy(sB, ytB1)
                    nc.vector.tensor_add(tmpA, ytA0, sA)
                    nc.vector.tensor_add(ystage[64:128, :], ytB0[0:D, :],
                                         sB[0:D, :])
                    nc.vector.tensor_add(sum1, ytB0[D : D + 1, :],
                                         sB[D : D + 1, :])
                    nc.vector.reciprocal(recA, tmpA[D : D + 1, :])
                    nc.vector.reciprocal(recB, sum1)
                    # broadcast 1/sum across partitions via a DRAM bounce
                    # (keeps the PE stream free of output-stage work)
                    recA_d = dp.tile([1, 512], f32, tag="recA_d",
                                     name=f"recAd{pr}_{I}")
                    recB_d = dp.tile([1, 512], f32, tag="recB_d",
                                     name=f"recBd{pr}_{I}")
                    rbc2 = op.tile([128, 512], f32, tag="rbc2",
                                   name=f"rbc2{pr}_{I}")
                    rbcA, rbcB = rbc2[0:64, :], rbc2[64:128, :]
                    nc.sync.dma_start(out=recA_d, in_=recA)
                    nc.sync.dma_start(out=recB_d, in_=recB)
                    nc.sync.dma_start(out=rbcA, in_=_bcast_ap(recA_d, 64))
                    nc.sync.dma_start(out=rbcB, in_=_bcast_ap(recB_d, 64))
                    nc.vector.tensor_mul(ystage[0:64, :], tmpA[0:D, :], rbcA)
                    nc.vector.tensor_mul(ystage[64:128, :], ystage[64:128, :],
                                         rbcB)
                    nc.sync.dma_start(
                        out=yT[pr * 128 : (pr + 1) * 128, isl], in_=ystage)

                # 1-stage software pipeline across the whole block: QK(k+1)
                # issues before PV(k) so the PE never sits behind a PV that
                # is waiting on exp
                items = [(pr, J) for pr in range(PAIRS) for J in range(nj)]
                emitted = 0
                done = 0

                def emit_qk(k):
                    pr, J = items[k]
                    if J == 0:
                        alloc_yt(pr)
                    qk_exp(pr, J)

                # lookahead-1 pipeline, deepened to 2 at pair boundaries so
                # the first PV of a new pair isn't reached while the DVE is
                # still releasing the previous pair's yt slots
                nxt = list(nxt)
                stride = max(1, len(items) // len(nxt)) if nxt else 0
                emitted = 0
                # constant lookahead-2 pipeline: QK/exp for items k+1 and k+2
                # are already in the stream when PV(k) issues, so neither a
                # PV wait nor a woven QKV group ever starves ScalarE
                for k in range(len(items)):
                    while emitted < min(k + 4, len(items)):
                        emit_qk(emitted)
                        emitted += 1
                    pr, J = items[k]
                    pv(pr, J)
                    if J == nj - 1:
                        out_stage(pr)
                    # weave next t-block's QKV groups into the PE stream
                    if nxt and (k + 1) % stride == 0:
                        fn, a, b = nxt.pop(0)
                        fn(a, b)
                for fn, a, b in nxt:
                    fn(a, b)

            # schedule: per t-block, QKV projection then attention I = tb
            # (causal: block I only needs k/v from t-blocks <= I)
            for g in range(4):
                qkv_group_v(0, g)
            for g in range(8):
                qkv_group_qk(0, g)
            for I in range(TB):
                nxt = []
                if I + 1 < TB:
                    load_x(I + 1)
                    nxt = [(qkv_group_qk, I + 1, g) for g in range(8)] + [
                        (qkv_group_v, I + 1, g) for g in range(4)
                    ]
                attn_block(I, nxt)
    nc.compile()
    return nc


def _bcast_ap(src_ap, nparts):
    """Partition-broadcast view of a [1, N] DRAM AP -> [nparts, N]."""
    import concourse.bass as bass

    return bass.AP(
        tensor=src_ap.tensor,
        offset=src_ap.offset,
        ap=[[0, nparts]] + list(src_ap.ap)[1:],
    )


def get_nc():
    if "nc" not in _cache:
        _cache["nc"] = _build_nc()
    return _cache["nc"]


def shard_inputs(x, w_attn, b_attn):
    """Full inputs -> per-core input maps (host-side slicing/transposition)."""
    x = np.asarray(x, dtype=np.float32)
    w = np.asarray(w_attn, dtype=np.float32)
    bb = np.asarray(b_attn, dtype=np.float32)
    in_maps = []
    for core in range(N_CORES):
        b, hg = core // 2, core % 2
        r0 = hg * CC  # first q row for this head group
        w_qk = np.ascontiguousarray(
            np.concatenate([w[r0 : r0 + CC, :], w[C + r0 : C + r0 + CC, :]], axis=0).T
        )
        w_v = np.ascontiguousarray(w[2 * C + r0 : 2 * C + r0 + CC, :].T)
        b_qk = np.stack(
            [bb[r0 + ct * 128 : r0 + (ct + 1) * 128] for ct in range(4)]
            + [bb[C + r0 + ct * 128 : C + r0 + (ct + 1) * 128] for ct in range(4)],
            axis=1,
        ).astype(np.float32)
        b_v = bb[2 * C + r0 : 2 * C + r0 + CC].reshape(1, CC).astype(np.float32)
        in_maps.append(
            {
                "xT": np.ascontiguousarray(x[b].T),
                "w_qk": w_qk,
                "w_v": w_v,
                "b_qk": np.ascontiguousarray(b_qk),
                "b_v": np.ascontiguousarray(b_v),
                "ones_d": np.ones((1, 128), dtype=np.float32),
            }
        )
    return in_maps


def run(in_maps, trace=False, **kw):
    from concourse import bass_utils

    nc = get_nc()
    return bass_utils.run_bass_kernel_spmd(
        nc, in_maps, core_ids=list(range(N_CORES)), trace=trace, **kw
    )


def gather_output(results):
    y = np.empty((B, T, E), dtype=np.float32)
    for core in range(N_CORES):
        b, hg = core // 2, core % 2
        y[b, :, hg * CC : (hg + 1) * CC] = results[core]["yT"].T
    return y


def kernel(x, w_attn, b_attn):
    in_maps = shard_inputs(x, w_attn, b_attn)
    res = run(in_maps, trace=False)
    return gather_output(res.results)



# revision 7
# speedup vs baseline: 1.0901x; 1.0901x over previous
"""Causal self-attention (B=4, T=2048, E=1024, H=16) on 8 trn2 NeuronCores.

Sharding: core c -> (batch b = c // 2, head-group hg = c % 2); each core owns
one batch element and 8 of the 16 heads (data parallel on B, tensor parallel
on heads).  No cross-core communication.

Per-core device program (SPMD, same NEFF on all 8 cores).  All 160 attention
items (I-block, head-pair, j-tile) run as ONE stream; the QKV projection
groups for block tb+1 are woven into the stream by a greedy scheduler that
keeps the PE fed without starving ScalarE (est-cost balance + hard deadlines
before each consuming unit).

  x / w in bf16 (same PE cost as fp32r, half the DMA bytes).
  q/k bias adds on DVE (tensor_scalar_add), v bias via K=1 ones matmul;
    ScalarE runs exp only.
  QK: row-tile pair, 2 heads per [128,1024] PSUM st tile (3 bufs).
  PV: full-128 contraction per head into one [65,1024] PSUM yt (row 64 =
    softmax denominator via a ones column in v).
  Causal: j-tiles past the diagonal are skipped; on-diagonal tiles are
    width-trimmed (QK moving, exp, PV moving) and the 128-wide boundary
    tile masked with a gpsimd affine_select after exp.
  Output: yt -> SBUF copy, reciprocal of row 64, gpsimd partition_broadcast,
    two DVE muls, DMA out of the DVE queue (keeps SP free for loads).
"""

import sys

sys.path.insert(0, "/opt/trn_rl_repo")

import numpy as np

N_CORES = 8
B, T, E = 4, 2048, 1024
H, D = 16, 64
C = E                 # q/k/v channel count (4th qkv chunk unused)
HPC = H // 2          # heads per core
CC = HPC * D          # per-core channels = 512
ES = E // 128         # 8 e-tiles (contraction)
TB = T // 512         # 4 t/i blocks of 512
NJ = T // 128         # 16 j-tiles of 128
PAIRS = HPC // 2      # 4 head pairs per core

ST_BUFS = 3
PT_BUFS = 6
LOOKAHEAD = 3
THRESH = 1500.0       # ns of ScalarE lead before weaving a group

_cache = {}


def _build_nc():
    import concourse.mybir as mybir
    import concourse.tile as tile
    from concourse import bacc

    f32 = mybir.dt.float32
    f32r = mybir.dt.float32r
    bf16 = mybir.dt.bfloat16
    Act = mybir.ActivationFunctionType
    is_ge = mybir.AluOpType.is_ge

    nc = bacc.Bacc("TRN2", target_bir_lowering=False, debug=False)

    xT = nc.dram_tensor("xT", [E, T], bf16, kind="ExternalInput").ap()
    w_qk = nc.dram_tensor("w_qk", [E, 2 * CC], bf16, kind="ExternalInput").ap()
    w_v = nc.dram_tensor("w_v", [E, CC], bf16, kind="ExternalInput").ap()
    b_qk = nc.dram_tensor("b_qk", [128, 8], f32, kind="ExternalInput").ap()
    b_v = nc.dram_tensor("b_v", [1, CC], f32r, kind="ExternalInput").ap()
    ones_d = nc.dram_tensor("ones_d", [1, 128], f32r, kind="ExternalInput").ap()
    yT = nc.dram_tensor("yT", [CC, T], f32, kind="ExternalOutput").ap()

    with tile.TileContext(nc) as tc:
        with (
            tc.tile_pool(name="persist", bufs=1) as pp,
            tc.tile_pool(name="psum", bufs=1, space="PSUM") as psp,
            tc.tile_pool(name="xpool", bufs=2) as xp,
            tc.tile_pool(name="qpool", bufs=2) as qp,
            tc.tile_pool(name="ptpool", bufs=1) as ptp,
            tc.tile_pool(name="opool", bufs=1) as op,
        ):
            # ---- persistent SBUF state ----
            k_sb = [pp.tile([128, T], f32r, name=f"k{p}") for p in range(PAIRS)]
            # v plus a ones column per head: [t-part, head, t-tile, 65]
            v1_sb = pp.tile([128, HPC, NJ, D + 1], f32r, name="v1")
            bqk_sb = pp.tile([128, 8], f32, name="bqk")
            bv_sb = pp.tile([1, CC], f32r, name="bv")
            ones_sb = pp.tile([1, 128], f32r, name="ones")
            wqk_h = [pp.tile([128, ES * 512], bf16, name=f"wqk{h}") for h in range(2)]
            wv_all = pp.tile([128, ES * 512], bf16, name="wv")

            nc.sync.dma_start(out=ones_sb, in_=ones_d)
            nc.sync.dma_start(out=v1_sb[:, :, :, D : D + 1],
                              in_=_bcast_ap(ones_d, 128))

            xs_tb = {}

            def load_x(tb):
                xt = xp.tile([128, ES * 512], bf16, tag="x", bufs=2,
                             name=f"x{tb}")
                nc.sync.dma_start(
                    out=xt,
                    in_=_mk_ap(xT, tb * 512, [[T, 128], [128 * T, ES], [1, 512]]),
                )
                xs_tb[tb] = xt

            # DMA order: small consts, x(0), q/k weights half 0 (pairs 0-1),
            # v weights, q/k half 1, x(1)
            nc.sync.dma_start(out=bqk_sb, in_=b_qk)
            nc.sync.dma_start(out=bv_sb, in_=b_v)
            load_x(0)
            nc.sync.dma_start(
                out=wqk_h[0],
                in_=_mk_ap(w_qk, 0, [[1024, 128], [128 * 1024, ES], [1, 512]]),
            )
            nc.sync.dma_start(
                out=wv_all,
                in_=_mk_ap(w_v, 0, [[512, 128], [128 * 512, ES], [1, 512]]),
            )
            nc.sync.dma_start(
                out=wqk_h[1],
                in_=_mk_ap(w_qk, 512, [[1024, 128], [128 * 1024, ES], [1, 512]]),
            )
            load_x(1)

            # ---- QKV projection groups ----
            q_tiles = {}

            def g_qk(tb, g):
                p, is_k = divmod(g, 2)
                h, sub = divmod(p, 2)
                xs = xs_tb[tb]
                ps = psp.tile([128, 512], f32, tag="st", bufs=ST_BUFS,
                              name=f"psqk{g}_{tb}")
                for e in range(ES):
                    c0 = e * 512 + 256 * sub + 128 * is_k
                    nc.tensor.matmul(
                        ps, wqk_h[h][:, c0 : c0 + 128],
                        xs[:, e * 512 : (e + 1) * 512],
                        start=(e == 0), stop=(e == ES - 1),
                    )
                if is_k:
                    dest = k_sb[p][:, tb * 512 : (tb + 1) * 512]
                else:
                    qt = qp.tile([128, 512], f32r, tag=f"q{p}", bufs=2,
                                 name=f"q{p}_{tb}")
                    q_tiles[(p, tb)] = qt
                    dest = qt
                nc.vector.tensor_scalar_add(dest, ps, bqk_sb[:, g : g + 1])

            def g_v(tb, k4):
                xs = xs_tb[tb]
                tt = tb * 4 + k4
                psv = psp.tile([128, 512], f32, tag="st", bufs=ST_BUFS,
                               name=f"psv{tt}")
                nc.tensor.matmul(
                    psv, ones_sb, bv_sb,
                    start=True, stop=False, skip_group_check=True,
                )
                for e in range(ES):
                    nc.tensor.matmul(
                        psv,
                        xs[:, e * 512 + 128 * k4 : e * 512 + 128 * (k4 + 1)],
                        wv_all[:, e * 512 : (e + 1) * 512],
                        start=False, stop=(e == ES - 1),
                        skip_group_check=True,
                    )
                nc.vector.tensor_copy(
                    v1_sb[:, :, tt, 0:D],
                    psv.rearrange("p (h d) -> p h d", d=D),
                )

            # ---- attention ----
            pts = {}
            yts = {}

            def qk_exp(I, pr, J):
                r = J - 4 * I
                w0 = 128 * r if r >= 1 else 0
                qt = q_tiles[(pr, I)]
                kt = k_sb[pr]
                jsl = slice(J * 128, (J + 1) * 128)
                st = psp.tile([128, 1024], f32, tag="st", bufs=ST_BUFS,
                              name=f"st{pr}_{I}_{J}")
                nc.tensor.matmul(
                    st[:, w0:512], kt[0:64, jsl], qt[0:64, w0:512],
                    tile_position=(0, 0),
                )
                nc.tensor.matmul(
                    st[:, 512 + w0 : 1024], kt[64:128, jsl], qt[64:128, w0:512],
                    tile_position=(64, 0),
                )
                pt = ptp.tile([128, 1024], f32r, tag="pt", bufs=PT_BUFS,
                              name=f"pt{pr}_{I}_{J}")
                Exp = Act.Exp
                if r < 1:
                    nc.scalar.activation(pt, st, Exp, scale=0.125)
                else:
                    nc.scalar.activation(pt[:, w0:512], st[:, w0:512], Exp,
                                         scale=0.125)
                    nc.scalar.activation(pt[:, 512 + w0 : 1024],
                                         st[:, 512 + w0 : 1024], Exp,
                                         scale=0.125)
                if r >= 0:  # mask the 128-wide boundary tile: keep col >= row
                    for off in (0, 512):
                        nc.gpsimd.affine_select(
                            out=pt[:, off + w0 : off + w0 + 128],
                            in_=pt[:, off + w0 : off + w0 + 128],
                            compare_op=is_ge,
                            fill=0.0,
                            base=0,
                            pattern=[[1, 128]],
                            channel_multiplier=-1,
                        )
                pts[(I, pr, J)] = pt

            def pv(I, pr, J):
                pt = pts.pop((I, pr, J))
                r = J - 4 * I
                w0 = 128 * r if r >= 1 else 0
                first, last = (J == 0), (J == 4 * I + 3)
                if first:
                    yts[pr] = psp.tile([D + 1, 1024], f32, tag="yt", bufs=1,
                                       name=f"yt{pr}_{I}")
                yt = yts[pr]
                nc.tensor.matmul(
                    yt[:, w0:512], v1_sb[:, 2 * pr, J, :], pt[:, w0:512],
                    start=first, stop=last, skip_group_check=True,
                )
                nc.tensor.matmul(
                    yt[:, 512 + w0 : 1024], v1_sb[:, 2 * pr + 1, J, :],
                    pt[:, 512 + w0 : 1024],
                    start=first, stop=last, skip_group_check=True,
                )

            def out_stage(I, pr):
                yt = yts.pop(pr)
                tmp = op.tile([D + 1, 1024], f32, tag="tmp", bufs=2,
                              name=f"tmp{pr}_{I}")
                nc.vector.tensor_copy(tmp, yt)   # frees the yt psum banks
                rec = op.tile([1, 1024], f32, tag="rec", bufs=2,
                              name=f"rec{pr}_{I}")
                nc.vector.reciprocal(rec, tmp[D : D + 1, :])
                rbc = op.tile([64, 1024], f32, tag="rbc", bufs=2,
                              name=f"rbc{pr}_{I}")
                nc.gpsimd.partition_broadcast(rbc, rec)
                ystage = op.tile([128, 512], f32, tag="ystage", bufs=2,
                                 name=f"ys{pr}_{I}")
                nc.vector.tensor_mul(ystage[0:64, :], tmp[0:D, 0:512],
                                     rbc[:, 0:512])
                nc.vector.tensor_mul(ystage[64:128, :], tmp[0:D, 512:1024],
                                     rbc[:, 512:1024])
                nc.sync.dma_start(
                    out=yT[pr * 128 : (pr + 1) * 128, I * 512 : (I + 1) * 512],
                    in_=ystage)

            # ---- schedule: one global item stream + greedy group weave ----
            units = [(I, p) for I in range(TB) for p in range(PAIRS)]
            uidx = {u: i for i, u in enumerate(units)}
            items = [(I, p, J) for (I, p) in units for J in range(4 * (I + 1))]

            def qk_pe(I, J):
                r = J - 4 * I
                w = 512 - (128 * r if r >= 1 else 0)
                return w * 0.4167 + 4.0

            def pv_pe(I, J):
                r = J - 4 * I
                w = 512 - (128 * r if r >= 1 else 0)
                return 2 * w * 0.4167

            def act_cost(I, J):
                r = J - 4 * I
                if r < 1:
                    return 1024 * 0.833 + 185
                w = 512 - 128 * r
                return 2 * (w * 0.833 + 185)

            GQK_PE = 8 * 512 * 0.4167
            GV_PE = 9 * 512 * 0.4167

            # group queue: (deadline unit, source block, fn, pe_cost)
            gq = []
            gq.append((0, 0, lambda: g_qk(0, 0), GQK_PE))
            gq.append((0, 0, lambda: g_qk(0, 1), GQK_PE))
            for p in range(1, PAIRS):
                gq.append((uidx[(0, p)], 0, lambda p=p: g_qk(0, 2 * p), GQK_PE))
                gq.append((uidx[(0, p)], 0,
                           lambda p=p: g_qk(0, 2 * p + 1), GQK_PE))
            for tb in range(1, TB):
                u0 = uidx[(tb, 0)]
                gq.append((u0, tb, lambda tb=tb: g_qk(tb, 0), GQK_PE))
                gq.append((u0, tb, lambda tb=tb: g_qk(tb, 1), GQK_PE))
                for k4 in range(4):
                    gq.append((u0, tb, lambda tb=tb, k4=k4: g_v(tb, k4), GV_PE))
                for p in range(1, PAIRS):
                    up = uidx[(tb, p)]
                    gq.append((up, tb,
                               lambda tb=tb, p=p: g_qk(tb, 2 * p), GQK_PE))
                    gq.append((up, tb,
                               lambda tb=tb, p=p: g_qk(tb, 2 * p + 1), GQK_PE))

            state = {"pe": 0.0, "act": 0.0, "gi": 0, "emitted": 0}
            seen_units = set()

            def emit_group():
                _, _, fn, pe = gq[state["gi"]]
                state["gi"] += 1
                fn()
                state["pe"] += pe

            def flush_deadlines(u):
                while state["gi"] < len(gq) and gq[state["gi"]][0] <= u:
                    emit_group()

            def emit_qk_item(k):
                I, p, J = items[k]
                u = uidx[(I, p)]
                if u not in seen_units:
                    seen_units.add(u)
                    if (I, p) == (1, 0):
                        load_x(2)
                    elif (I, p) == (2, 0):
                        load_x(3)
                flush_deadlines(u)
                qk_exp(I, p, J)
                state["act"] += act_cost(I, J)
                state["pe"] += qk_pe(I, J)

            for k in range(len(items)):
                while state["emitted"] < min(k + 1 + LOOKAHEAD, len(items)):
                    emit_qk_item(state["emitted"])
                    state["emitted"] += 1
                I, p, J = items[k]
                if k == 0:  # block-0 v groups: after first QKs, before first PV
                    for k4 in range(4):
                        g_v(0, k4)
                        state["pe"] += GV_PE
                pv(I, p, J)
                state["pe"] += pv_pe(I, J)
                if J == 4 * I + 3:
                    out_stage(I, p)
                while (state["gi"] < len(gq) and gq[state["gi"]][1] <= I + 1
                       and state["act"] - state["pe"] > THRESH):
                    emit_group()
            while state["gi"] < len(gq):
                emit_group()
    nc.compile()
    return nc


def _mk_ap(src_ap, offset, dims):
    """Raw strided view of a DRAM tensor (strides/offset in elements)."""
    import concourse.bass as bass

    return bass.AP(tensor=src_ap.tensor, offset=offset, ap=dims)


def _bcast_ap(src_ap, nparts):
    """Partition-broadcast view of a [1, N] DRAM AP -> [nparts, N]."""
    import concourse.bass as bass

    return bass.AP(
        tensor=src_ap.tensor,
        offset=src_ap.offset,
        ap=[[0, nparts]] + list(src_ap.ap)[1:],
    )


def get_nc():
    if "nc" not in _cache:
        _cache["nc"] = _build_nc()
    return _cache["nc"]


def shard_inputs(x, w_attn, b_attn):
    """Full inputs -> per-core input maps (host-side slicing/transposition)."""
    import ml_dtypes

    bf16 = ml_dtypes.bfloat16
    x = np.asarray(x, dtype=np.float32)
    w = np.asarray(w_attn, dtype=np.float32)
    bb = np.asarray(b_attn, dtype=np.float32)
    in_maps = []
    for core in range(N_CORES):
        b, hg = core // 2, core % 2
        r0 = hg * CC  # first q row for this head group
        # pair-interleaved q/k weight columns: [q_p0|k_p0|q_p1|k_p1|...]
        blocks = []
        bcols = []
        for p in range(PAIRS):
            blocks.append(w[r0 + 128 * p : r0 + 128 * (p + 1), :].T)
            blocks.append(w[C + r0 + 128 * p : C + r0 + 128 * (p + 1), :].T)
            bcols.append(bb[r0 + 128 * p : r0 + 128 * (p + 1)])
            bcols.append(bb[C + r0 + 128 * p : C + r0 + 128 * (p + 1)])
        w_qk = np.ascontiguousarray(np.concatenate(blocks, axis=1).astype(bf16))
        b_qk = np.ascontiguousarray(np.stack(bcols, axis=1).astype(np.float32))
        w_v = np.ascontiguousarray(
            w[2 * C + r0 : 2 * C + r0 + CC, :].T.astype(bf16))
        b_v = bb[2 * C + r0 : 2 * C + r0 + CC].reshape(1, CC).astype(np.float32)
        in_maps.append(
            {
                "xT": np.ascontiguousarray(x[b].T.astype(bf16)),
                "w_qk": w_qk,
                "w_v": w_v,
                "b_qk": b_qk,
                "b_v": np.ascontiguousarray(b_v),
                "ones_d": np.ones((1, 128), dtype=np.float32),
            }
        )
    return in_maps


def run(in_maps, trace=False, **kw):
    from concourse import bass_utils

    nc = get_nc()
    return bass_utils.run_bass_kernel_spmd(
        nc, in_maps, core_ids=list(range(N_CORES)), trace=trace, **kw
    )


def gather_output(results):
    y = np.empty((B, T, E), dtype=np.float32)
    for core in range(N_CORES):
        b, hg = core // 2, core % 2
        y[b, :, hg * CC : (hg + 1) * CC] = results[core]["yT"].T
    return y


def kernel(x, w_attn, b_attn):
    in_maps = shard_inputs(x, w_attn, b_attn)
    res = run(in_maps, trace=False)
    return gather_output(res.results)


# revision 23
# speedup vs baseline: 1.2034x; 1.1039x over previous
"""Causal self-attention (B=4, T=2048, E=1024, H=16) on 8 trn2 NeuronCores.

Sharding: core c -> (batch b = c // 2, head-group hg = c % 2); each core owns
one batch element and 8 of the 16 heads (data parallel on B, tensor parallel
on heads).  No cross-core communication.

Per-core device program (SPMD, same NEFF on all 8 cores).  All 160 attention
items (I-block, head-pair, j-tile) run as ONE stream; the QKV projection
groups for block tb+1 are woven into the stream by a greedy scheduler that
keeps the PE fed without starving ScalarE (est-cost balance + hard deadlines
before each consuming unit).

  x / w in bf16 (same PE cost as fp32r, half the DMA bytes).
  q/k bias adds on DVE (tensor_scalar_add), v bias via K=1 ones matmul;
    ScalarE runs exp only.
  QK: row-tile pair, 2 heads per [128,1024] PSUM st tile (3 bufs).
  PV: full-128 contraction per head into one [65,1024] PSUM yt (row 64 =
    softmax denominator via a ones column in v).
  Causal: j-tiles past the diagonal are skipped; on-diagonal tiles are
    width-trimmed (QK moving, exp, PV moving) and the 128-wide boundary
    tile masked with a gpsimd affine_select after exp.
  Output: yt -> SBUF copy, reciprocal of row 64, gpsimd partition_broadcast,
    two DVE muls, DMA out of the DVE queue (keeps SP free for loads).
"""

import sys

sys.path.insert(0, "/opt/trn_rl_repo")

import numpy as np

N_CORES = 8
B, T, E = 4, 2048, 1024
H, D = 16, 64
C = E                 # q/k/v channel count (4th qkv chunk unused)
HPC = H // 2          # heads per core
CC = HPC * D          # per-core channels = 512
ES = E // 128         # 8 e-tiles (contraction)
TB = T // 512         # 4 t/i blocks of 512
NJ = T // 128         # 16 j-tiles of 128
PAIRS = HPC // 2      # 4 head pairs per core

CCV = HPC * (D + 1)   # v channels incl. a ones column per head = 520

ST_BUFS = 3
PT_BUFS = 6
LOOKAHEAD = 3

_cache = {}


def _build_nc():
    import concourse.mybir as mybir
    import concourse.tile as tile
    from concourse import bacc

    f32 = mybir.dt.float32
    f32r = mybir.dt.float32r
    bf16 = mybir.dt.bfloat16
    Act = mybir.ActivationFunctionType
    is_ge = mybir.AluOpType.is_ge

    nc = bacc.Bacc("TRN2", target_bir_lowering=False, debug=False)

    xT = nc.dram_tensor("xT", [E, T], bf16, kind="ExternalInput").ap()
    w_qk = nc.dram_tensor("w_qk", [E, 2 * CC], bf16, kind="ExternalInput").ap()
    w_v = nc.dram_tensor("w_v", [E, CC], bf16, kind="ExternalInput").ap()
    b_qk = nc.dram_tensor("b_qk", [128, 8], f32, kind="ExternalInput").ap()
    b_v = nc.dram_tensor("b_v", [1, CC], f32r, kind="ExternalInput").ap()
    ones_d = nc.dram_tensor("ones_d", [1, 128], f32r, kind="ExternalInput").ap()
    yT = nc.dram_tensor("yT", [CC, T], f32, kind="ExternalOutput").ap()

    with tile.TileContext(nc) as tc:
        with (
            tc.tile_pool(name="persist", bufs=1) as pp,
            tc.tile_pool(name="psum", bufs=1, space="PSUM") as psp,
            tc.tile_pool(name="xpool", bufs=2) as xp,
            tc.tile_pool(name="qpool", bufs=2) as qp,
            tc.tile_pool(name="ptpool", bufs=1) as ptp,
            tc.tile_pool(name="opool", bufs=1) as op,
        ):
            # ---- persistent SBUF state ----
            k_sb = [pp.tile([128, T], f32r, name=f"k{p}") for p in range(PAIRS)]
            # v plus a ones column per head: [t-part, head, t-tile, 65]
            v1_sb = pp.tile([128, HPC, NJ, D + 1], f32r, name="v1")
            bqk_sb = pp.tile([128, 8], f32, name="bqk")
            bv_sb = pp.tile([1, CC], f32r, name="bv")
            ones_sb = pp.tile([1, 128], f32r, name="ones")
            wqk_h = [pp.tile([128, ES * 512], bf16, name=f"wqk{h}") for h in range(2)]
            wv_all = pp.tile([128, ES * 512], bf16, name="wv")

            # softmax-denominator ones column of v1: memset rejects f32r, so
            # stage a broadcast ones tile and strided-copy it in on DVE
            ones128 = pp.tile([128, 128], f32r, name="ones128")

            xs_tb = {}

            def load_x(tb):
                xt = xp.tile([128, ES * 512], bf16, tag="x", bufs=2,
                             name=f"x{tb}")
                nc.sync.dma_start(
                    out=xt,
                    in_=_mk_ap(xT, tb * 512, [[T, 128], [128 * T, ES], [1, 512]]),
                )
                xs_tb[tb] = xt

            # DMA order: small consts, x(0), q/k weights for pair 0, then
            # pair 1, v weights (with built-in ones column), q/k half 1, x(1)
            nc.sync.dma_start(out=bqk_sb, in_=b_qk)
            nc.sync.dma_start(out=bv_sb, in_=b_v)
            nc.sync.dma_start(out=ones_sb, in_=ones_d)
            nc.sync.dma_start(out=ones128, in_=_bcast_ap(ones_d, 128))
            nc.vector.tensor_copy(v1_sb[:, :, :, D : D + 1], ones128)
            load_x(0)
            wqk0_e = wqk_h[0].rearrange("p (e c) -> p e c", e=ES)
            for sub in range(2):  # pair 0 first so attention starts earliest
                nc.sync.dma_start(
                    out=wqk0_e[:, :, 256 * sub : 256 * sub + 256],
                    in_=_mk_ap(w_qk, 256 * sub,
                               [[1024, 128], [128 * 1024, ES], [1, 256]]),
                )
            nc.sync.dma_start(
                out=wv_all,
                in_=_mk_ap(w_v, 0, [[512, 128], [128 * 512, ES], [1, 512]]),
            )
            nc.sync.dma_start(
                out=wqk_h[1],
                in_=_mk_ap(w_qk, 512, [[1024, 128], [128 * 1024, ES], [1, 512]]),
            )
            load_x(1)

            # ---- QKV projection groups ----
            q_tiles = {}

            def g_qk(tb, g):
                p, is_k = divmod(g, 2)
                h, sub = divmod(p, 2)
                xs = xs_tb[tb]
                ps = psp.tile([128, 512], f32, tag="st", bufs=ST_BUFS,
                              name=f"psqk{g}_{tb}")
                for e in range(ES):
                    c0 = e * 512 + 256 * sub + 128 * is_k
                    nc.tensor.matmul(
                        ps, wqk_h[h][:, c0 : c0 + 128],
                        xs[:, e * 512 : (e + 1) * 512],
                        start=(e == 0), stop=(e == ES - 1),
                    )
                if is_k:
                    dest = k_sb[p][:, tb * 512 : (tb + 1) * 512]
                else:
                    qt = qp.tile([128, 512], f32r, tag=f"q{p}", bufs=2,
                                 name=f"q{p}_{tb}")
                    q_tiles[(p, tb)] = qt
                    dest = qt
                nc.vector.tensor_scalar_add(dest, ps, bqk_sb[:, g : g + 1])

            def g_v(tb, k4):
                xs = xs_tb[tb]
                tt = tb * 4 + k4
                psv = psp.tile([128, 512], f32, tag="st", bufs=ST_BUFS,
                               name=f"psv{tt}")
                nc.tensor.matmul(
                    psv, ones_sb, bv_sb,
                    start=True, stop=False, skip_group_check=True,
                )
                for e in range(ES):
                    nc.tensor.matmul(
                        psv,
                        xs[:, e * 512 + 128 * k4 : e * 512 + 128 * (k4 + 1)],
                        wv_all[:, e * 512 : (e + 1) * 512],
                        start=False, stop=(e == ES - 1),
                        skip_group_check=True,
                    )
                nc.vector.tensor_copy(
                    v1_sb[:, :, tt, 0:D],
                    psv.rearrange("p (h d) -> p h d", d=D),
                )

            # ---- attention ----
            pts = {}
            yts = {}

            def qk_exp(I, pr, J):
                r = J - 4 * I
                w0 = 128 * r if r >= 1 else 0
                qt = q_tiles[(pr, I)]
                kt = k_sb[pr]
                jsl = slice(J * 128, (J + 1) * 128)
                st = psp.tile([128, 1024], f32, tag="st", bufs=ST_BUFS,
                              name=f"st{pr}_{I}_{J}")
                nc.tensor.matmul(
                    st[:, w0:512], kt[0:64, jsl], qt[0:64, w0:512],
                    tile_position=(0, 0),
                )
                nc.tensor.matmul(
                    st[:, 512 + w0 : 1024], kt[64:128, jsl], qt[64:128, w0:512],
                    tile_position=(64, 0),
                )
                pt = ptp.tile([128, 1024], f32r, tag="pt", bufs=PT_BUFS,
                              name=f"pt{pr}_{I}_{J}")
                Exp = Act.Exp
                if r < 1:
                    nc.scalar.activation(pt, st, Exp, scale=0.125)
                else:
                    nc.scalar.activation(pt[:, w0:512], st[:, w0:512], Exp,
                                         scale=0.125)
                    nc.scalar.activation(pt[:, 512 + w0 : 1024],
                                         st[:, 512 + w0 : 1024], Exp,
                                         scale=0.125)
                if r >= 0:  # mask the 128-wide boundary tile: keep col >= row
                    for off in (0, 512):
                        nc.gpsimd.affine_select(
                            out=pt[:, off + w0 : off + w0 + 128],
                            in_=pt[:, off + w0 : off + w0 + 128],
                            compare_op=is_ge,
                            fill=0.0,
                            base=0,
                            pattern=[[1, 128]],
                            channel_multiplier=-1,
                        )
                pts[(I, pr, J)] = pt

            def pv(I, pr, J):
                pt = pts.pop((I, pr, J))
                r = J - 4 * I
                w0 = 128 * r if r >= 1 else 0
                first, last = (J == 0), (J == 4 * I + 3)
                if first:
                    yts[pr] = (
                        psp.tile([D + 1, 512], f32, tag="ytA", bufs=1,
                                 name=f"ytA{pr}_{I}"),
                        psp.tile([D + 1, 512], f32, tag="ytB", bufs=1,
                                 name=f"ytB{pr}_{I}"),
                    )
                ytA, ytB = yts[pr]
                nc.tensor.matmul(
                    ytA[:, w0:512], v1_sb[:, 2 * pr, J, :], pt[:, w0:512],
                    start=first, stop=last, skip_group_check=True,
                )
                nc.tensor.matmul(
                    ytB[:, w0:512], v1_sb[:, 2 * pr + 1, J, :],
                    pt[:, 512 + w0 : 1024],
                    start=first, stop=last, skip_group_check=True,
                )

            def out_stage(I, pr):
                ytA, ytB = yts.pop(pr)
                tmpA = op.tile([D + 1, 512], f32, tag="tmpA", bufs=2,
                               name=f"tmpA{pr}_{I}")
                tmpB = op.tile([D + 1, 512], f32, tag="tmpB", bufs=2,
                               name=f"tmpB{pr}_{I}")
                nc.vector.tensor_copy(tmpA, ytA)   # frees head-A psum bank
                nc.vector.tensor_copy(tmpB, ytB)   # frees head-B psum bank
                rec = op.tile([1, 1024], f32, tag="rec", bufs=2,
                              name=f"rec{pr}_{I}")
                nc.vector.reciprocal(rec[:, 0:512], tmpA[D : D + 1, :])
                nc.vector.reciprocal(rec[:, 512:1024], tmpB[D : D + 1, :])
                rbc = op.tile([64, 1024], f32, tag="rbc", bufs=2,
                              name=f"rbc{pr}_{I}")
                nc.gpsimd.partition_broadcast(rbc, rec)
                ystage = op.tile([128, 512], f32, tag="ystage", bufs=2,
                                 name=f"ys{pr}_{I}")
                nc.vector.tensor_mul(ystage[0:64, :], tmpA[0:D, :],
                                     rbc[:, 0:512])
                nc.vector.tensor_mul(ystage[64:128, :], tmpB[0:D, :],
                                     rbc[:, 512:1024])
                nc.sync.dma_start(
                    out=yT[pr * 128 : (pr + 1) * 128, I * 512 : (I + 1) * 512],
                    in_=ystage)

            # ---- schedule: one global item stream with spread group weave ----
            # Each unit (I, pr) = all j-tiles of one head-pair/query-block.
            # QKV groups are pre-assigned to the unit ONE consumer earlier and
            # interleaved between its PV calls at an even stride, so the PE
            # stream always has the next unit's inputs ready and ScalarE's
            # exp queue never drains at unit/block boundaries.
            units = [(I, p) for I in range(TB) for p in range(PAIRS)]
            uidx = {u: i for i, u in enumerate(units)}
            items = [(I, p, J) for (I, p) in units for J in range(4 * (I + 1))]

            weave = {u: [] for u in range(len(units))}
            # q/k groups of pair p>=1 -> woven into the previous pair's unit
            for tb in range(TB):
                for p in range(1, PAIRS):
                    weave[uidx[(tb, p)] - 1] += [
                        lambda tb=tb, p=p: g_qk(tb, 2 * p),
                        lambda tb=tb, p=p: g_qk(tb, 2 * p + 1),
                    ]
            # block tb+1 pair-0 q/k + all v groups -> spread over block tb
            for tb in range(1, TB):
                weave[uidx[(tb - 1, 0)]] += [
                    lambda tb=tb: g_v(tb, 0), lambda tb=tb: g_v(tb, 1)]
                weave[uidx[(tb - 1, 1)]] += [
                    lambda tb=tb: g_v(tb, 2), lambda tb=tb: g_v(tb, 3)]
                weave[uidx[(tb - 1, 2)]] += [
                    lambda tb=tb: g_qk(tb, 0), lambda tb=tb: g_qk(tb, 1)]

            # deadline fallback (correctness): groups not yet woven when a
            # unit's first QK must be emitted are flushed here, in order
            gq_order = []
            for u in range(len(units)):
                for fn in weave[u]:
                    gq_order.append(fn)
            woven = set()

            state = {"emitted": 0}
            seen_units = set()
            unit_first_item = {}
            pos = 0
            for k, (I, p, J) in enumerate(items):
                if J == 0:
                    unit_first_item[uidx[(I, p)]] = k

            def emit_weave(fn):
                if id(fn) not in woven:
                    woven.add(id(fn))
                    fn()

            def flush_for_unit(u):
                # everything assigned to earlier units must be in-stream
                for uu in range(u):
                    for fn in weave[uu]:
                        emit_weave(fn)

            def emit_qk_item(k):
                I, p, J = items[k]
                u = uidx[(I, p)]
                if u not in seen_units:
                    seen_units.add(u)
                    if (I, p) == (1, 0):
                        load_x(2)
                    elif (I, p) == (2, 0):
                        load_x(3)
                    flush_for_unit(u)
                qk_exp(I, p, J)

            g_qk(0, 0)
            g_qk(0, 1)
            for k in range(len(items)):
                while state["emitted"] < min(k + 1 + LOOKAHEAD, len(items)):
                    emit_qk_item(state["emitted"])
                    state["emitted"] += 1
                I, p, J = items[k]
                u = uidx[(I, p)]
                if k == 0:  # block-0 v groups: after first QKs, before first PV
                    for k4 in range(4):
                        g_v(0, k4)
                pv(I, p, J)
                if J == 4 * I + 3:
                    out_stage(I, p)
                # spread this unit's weave list across its items, finishing
                # LOOKAHEAD items before the unit ends
                wl = weave[u]
                if wl:
                    n_items = 4 * (I + 1)
                    slots = max(1, n_items - LOOKAHEAD)
                    j_in = k - unit_first_item[u]
                    done = min(len(wl), (j_in + 1) * len(wl) // slots
                               if slots > 1 else len(wl))
                    for fn in wl[:done]:
                        emit_weave(fn)
            for fn in gq_order:
                emit_weave(fn)
    nc.compile()
    return nc


def _mk_ap(src_ap, offset, dims):
    """Raw strided view of a DRAM tensor (strides/offset in elements)."""
    import concourse.bass as bass

    return bass.AP(tensor=src_ap.tensor, offset=offset, ap=dims)


def _bcast_ap(src_ap, nparts):
    """Partition-broadcast view of a [1, N] DRAM AP -> [nparts, N]."""
    import concourse.bass as bass

    return bass.AP(
        tensor=src_ap.tensor,
        offset=src_ap.offset,
        ap=[[0, nparts]] + list(src_ap.ap)[1:],
    )


def get_nc():
    if "nc" not in _cache:
        _cache["nc"] = _build_nc()
    return _cache["nc"]


def shard_inputs(x, w_attn, b_attn):
    """Full inputs -> per-core input maps (host-side slicing/transposition)."""
    import ml_dtypes

    bf16 = ml_dtypes.bfloat16
    x = np.asarray(x, dtype=np.float32)
    w = np.asarray(w_attn, dtype=np.float32)
    bb = np.asarray(b_attn, dtype=np.float32)
    in_maps = []
    for core in range(N_CORES):
        b, hg = core // 2, core % 2
        r0 = hg * CC  # first q row for this head group
        # pair-interleaved q/k weight columns: [q_p0|k_p0|q_p1|k_p1|...]
        blocks = []
        bcols = []
        for p in range(PAIRS):
            blocks.append(w[r0 + 128 * p : r0 + 128 * (p + 1), :].T)
            blocks.append(w[C + r0 + 128 * p : C + r0 + 128 * (p + 1), :].T)
            bcols.append(bb[r0 + 128 * p : r0 + 128 * (p + 1)])
            bcols.append(bb[C + r0 + 128 * p : C + r0 + 128 * (p + 1)])
        w_qk = np.ascontiguousarray(np.concatenate(blocks, axis=1).astype(bf16))
        b_qk = np.ascontiguousarray(np.stack(bcols, axis=1).astype(np.float32))
        w_v = np.ascontiguousarray(
            w[2 * C + r0 : 2 * C + r0 + CC, :].T.astype(bf16))
        b_v = bb[2 * C + r0 : 2 * C + r0 + CC].reshape(1, CC).astype(np.float32)
        in_maps.append(
            {
                "xT": np.ascontiguousarray(x[b].T.astype(bf16)),
                "w_qk": w_qk,
                "w_v": w_v,
                "b_qk": b_qk,
                "b_v": np.ascontiguousarray(b_v),
                "ones_d": np.ones((1, 128), dtype=np.float32),
            }
        )
    return in_maps


def run(in_maps, trace=False, **kw):
    from concourse import bass_utils

    nc = get_nc()
    return bass_utils.run_bass_kernel_spmd(
        nc, in_maps, core_ids=list(range(N_CORES)), trace=trace, **kw
    )


def gather_output(results):
    y = np.empty((B, T, E), dtype=np.float32)
    for core in range(N_CORES):
        b, hg = core // 2, core % 2
        y[b, :, hg * CC : (hg + 1) * CC] = results[core]["yT"].T
    return y


def kernel(x, w_attn, b_attn):
    in_maps = shard_inputs(x, w_attn, b_attn)
    res = run(in_maps, trace=False)
    return gather_output(res.results)


# revision 26
# speedup vs baseline: 1.2240x; 1.0171x over previous
"""Causal self-attention (B=4, T=2048, E=1024, H=16) on 8 trn2 NeuronCores.

Sharding: core c -> (batch b = c // 2, head-group hg = c % 2); each core owns
one batch element and 8 of the 16 heads (data parallel on B, tensor parallel
on heads).  No cross-core communication.

Per-core device program (SPMD, same NEFF on all 8 cores).  All 160 attention
items (I-block, head-pair, j-tile) run as ONE stream; the QKV projection
groups for block tb+1 are woven into the stream by a greedy scheduler that
keeps the PE fed without starving ScalarE (est-cost balance + hard deadlines
before each consuming unit).

  x / w in bf16 (same PE cost as fp32r, half the DMA bytes).
  q/k bias adds on DVE (tensor_scalar_add), v bias via K=1 ones matmul;
    ScalarE runs exp only.
  QK: row-tile pair, 2 heads per [128,1024] PSUM st tile (3 bufs).
  PV: full-128 contraction per head into one [65,1024] PSUM yt (row 64 =
    softmax denominator via a ones column in v).
  Causal: j-tiles past the diagonal are skipped; on-diagonal tiles are
    width-trimmed (QK moving, exp, PV moving) and the 128-wide boundary
    tile masked with a gpsimd affine_select after exp.
  Output: yt -> SBUF copy, reciprocal of row 64, gpsimd partition_broadcast,
    two DVE muls, DMA out of the DVE queue (keeps SP free for loads).
"""

import sys

sys.path.insert(0, "/opt/trn_rl_repo")

import numpy as np

N_CORES = 8
B, T, E = 4, 2048, 1024
H, D = 16, 64
C = E                 # q/k/v channel count (4th qkv chunk unused)
HPC = H // 2          # heads per core
CC = HPC * D          # per-core channels = 512
ES = E // 128         # 8 e-tiles (contraction)
TB = T // 512         # 4 t/i blocks of 512
NJ = T // 128         # 16 j-tiles of 128
PAIRS = HPC // 2      # 4 head pairs per core

CCV = HPC * (D + 1)   # v channels incl. a ones column per head = 520

ST_BUFS = 3
PT_BUFS = 6
LOOKAHEAD = 3

EMIT_LOG = {"ACT": [], "PE": []}  # emission-order labels, for trace analysis

_cache = {}


def _build_nc():
    import concourse.mybir as mybir
    import concourse.tile as tile
    from concourse import bacc

    f32 = mybir.dt.float32
    f32r = mybir.dt.float32r
    bf16 = mybir.dt.bfloat16
    Act = mybir.ActivationFunctionType
    is_ge = mybir.AluOpType.is_ge

    nc = bacc.Bacc("TRN2", target_bir_lowering=False, debug=False)

    xT = nc.dram_tensor("xT", [E, T], bf16, kind="ExternalInput").ap()
    w_qk = nc.dram_tensor("w_qk", [E, 2 * CC], bf16, kind="ExternalInput").ap()
    w_v = nc.dram_tensor("w_v", [E, CC], bf16, kind="ExternalInput").ap()
    b_qk = nc.dram_tensor("b_qk", [128, 8], f32, kind="ExternalInput").ap()
    b_v = nc.dram_tensor("b_v", [1, CC], f32r, kind="ExternalInput").ap()
    ones_d = nc.dram_tensor("ones_d", [1, 128], f32r, kind="ExternalInput").ap()
    yT = nc.dram_tensor("yT", [CC, T], f32, kind="ExternalOutput").ap()

    with tile.TileContext(nc) as tc:
        with (
            tc.tile_pool(name="persist", bufs=1) as pp,
            tc.tile_pool(name="psum", bufs=1, space="PSUM") as psp,
            tc.tile_pool(name="xpool", bufs=2) as xp,
            tc.tile_pool(name="qpool", bufs=2) as qp,
            tc.tile_pool(name="ptpool", bufs=1) as ptp,
            tc.tile_pool(name="opool", bufs=1) as op,
        ):
            # ---- persistent SBUF state ----
            k_sb = [pp.tile([128, T], f32r, name=f"k{p}") for p in range(PAIRS)]
            # v plus a ones column per head: [t-part, head, t-tile, 65]
            v1_sb = pp.tile([128, HPC, NJ, D + 1], f32r, name="v1")
            bqk_sb = pp.tile([128, 8], f32, name="bqk")
            bv_sb = pp.tile([1, CC], f32r, name="bv")
            ones_sb = pp.tile([1, 128], f32r, name="ones")
            wqk_h = [pp.tile([128, ES * 512], bf16, name=f"wqk{h}") for h in range(2)]
            wv_all = pp.tile([128, ES * 512], bf16, name="wv")

            # softmax-denominator ones column of v1: memset rejects f32r, so
            # stage a broadcast ones tile and strided-copy it in on DVE
            ones128 = pp.tile([128, 128], f32r, name="ones128")

            xs_tb = {}

            def load_x(tb):
                xt = xp.tile([128, ES * 512], bf16, tag="x", bufs=2,
                             name=f"x{tb}")
                nc.sync.dma_start(
                    out=xt,
                    in_=_mk_ap(xT, tb * 512, [[T, 128], [128 * T, ES], [1, 512]]),
                )
                xs_tb[tb] = xt

            # DMA order: x(0) + pair-0 q/k weights first (attention starts
            # earliest), then consts, v weights, pair-1 weights, x(1), rest
            load_x(0)
            wqk0_e = wqk_h[0].rearrange("p (e c) -> p e c", e=ES)
            nc.sync.dma_start(
                out=wqk0_e[:, :, 0:256],
                in_=_mk_ap(w_qk, 0, [[1024, 128], [128 * 1024, ES], [1, 256]]),
            )
            nc.sync.dma_start(out=bqk_sb, in_=b_qk)
            nc.sync.dma_start(out=bv_sb, in_=b_v)
            nc.sync.dma_start(out=ones_sb, in_=ones_d)
            nc.sync.dma_start(out=ones128, in_=_bcast_ap(ones_d, 128))
            nc.vector.tensor_copy(v1_sb[:, :, :, D : D + 1], ones128)
            nc.sync.dma_start(
                out=wv_all,
                in_=_mk_ap(w_v, 0, [[512, 128], [128 * 512, ES], [1, 512]]),
            )
            nc.sync.dma_start(
                out=wqk0_e[:, :, 256:512],
                in_=_mk_ap(w_qk, 256,
                           [[1024, 128], [128 * 1024, ES], [1, 256]]),
            )
            load_x(1)
            nc.sync.dma_start(
                out=wqk_h[1],
                in_=_mk_ap(w_qk, 512, [[1024, 128], [128 * 1024, ES], [1, 512]]),
            )

            # ---- QKV projection groups ----
            q_tiles = {}

            def g_qk(tb, g):
                EMIT_LOG["PE"] += [f"gqk{tb}_{g}.e{e}" for e in range(ES)]
                p, is_k = divmod(g, 2)
                h, sub = divmod(p, 2)
                xs = xs_tb[tb]
                ps = psp.tile([128, 512], f32, tag="st", bufs=ST_BUFS,
                              name=f"psqk{g}_{tb}")
                for e in range(ES):
                    c0 = e * 512 + 256 * sub + 128 * is_k
                    nc.tensor.matmul(
                        ps, wqk_h[h][:, c0 : c0 + 128],
                        xs[:, e * 512 : (e + 1) * 512],
                        start=(e == 0), stop=(e == ES - 1),
                    )
                if is_k:
                    dest = k_sb[p][:, tb * 512 : (tb + 1) * 512]
                else:
                    qt = qp.tile([128, 512], f32r, tag=f"q{p}", bufs=2,
                                 name=f"q{p}_{tb}")
                    q_tiles[(p, tb)] = qt
                    dest = qt
                nc.vector.tensor_scalar_add(dest, ps, bqk_sb[:, g : g + 1])

            def g_v(tb, k4):
                EMIT_LOG["PE"] += [f"gv{tb}_{k4}.b"] + [
                    f"gv{tb}_{k4}.e{e}" for e in range(ES)]
                xs = xs_tb[tb]
                tt = tb * 4 + k4
                psv = psp.tile([128, 512], f32, tag="st", bufs=ST_BUFS,
                               name=f"psv{tt}")
                nc.tensor.matmul(
                    psv, ones_sb, bv_sb,
                    start=True, stop=False, skip_group_check=True,
                )
                for e in range(ES):
                    nc.tensor.matmul(
                        psv,
                        xs[:, e * 512 + 128 * k4 : e * 512 + 128 * (k4 + 1)],
                        wv_all[:, e * 512 : (e + 1) * 512],
                        start=False, stop=(e == ES - 1),
                        skip_group_check=True,
                    )
                nc.vector.tensor_copy(
                    v1_sb[:, :, tt, 0:D],
                    psv.rearrange("p (h d) -> p h d", d=D),
                )

            # ---- attention ----
            pts = {}
            yts = {}

            def qk_exp(I, pr, J):
                EMIT_LOG["PE"] += [f"qk{I}{pr}{J}.A", f"qk{I}{pr}{J}.B"]
                r = J - 4 * I
                EMIT_LOG["ACT"] += [f"exp{I}{pr}{J}"]
                w0 = 128 * r if r >= 1 else 0
                qt = q_tiles[(pr, I)]
                kt = k_sb[pr]
                jsl = slice(J * 128, (J + 1) * 128)
                st = psp.tile([128, 1024], f32, tag="st", bufs=ST_BUFS,
                              name=f"st{pr}_{I}_{J}")
                nc.tensor.matmul(
                    st[:, w0:512], kt[0:64, jsl], qt[0:64, w0:512],
                    tile_position=(0, 0),
                )
                nc.tensor.matmul(
                    st[:, 512 + w0 : 1024], kt[64:128, jsl], qt[64:128, w0:512],
                    tile_position=(64, 0),
                )
                pt = ptp.tile([128, 1024], f32r, tag="pt", bufs=PT_BUFS,
                              name=f"pt{pr}_{I}_{J}")
                Exp = Act.Exp
                if r < 1:
                    nc.scalar.activation(pt, st, Exp, scale=0.125)
                else:  # one activation over both heads' trimmed windows
                    pt2 = pt.rearrange("p (h w) -> p h w", h=2)
                    st2 = st.rearrange("p (h w) -> p h w", h=2)
                    nc.scalar.activation(pt2[:, :, w0:512], st2[:, :, w0:512],
                                         Exp, scale=0.125)
                if r >= 0:  # mask the 128-wide boundary tile: keep col >= row
                    for off in (0, 512):
                        nc.gpsimd.affine_select(
                            out=pt[:, off + w0 : off + w0 + 128],
                            in_=pt[:, off + w0 : off + w0 + 128],
                            compare_op=is_ge,
                            fill=0.0,
                            base=0,
                            pattern=[[1, 128]],
                            channel_multiplier=-1,
                        )
                pts[(I, pr, J)] = pt

            def pv(I, pr, J):
                EMIT_LOG["PE"] += [f"pv{I}{pr}{J}.A", f"pv{I}{pr}{J}.B"]
                pt = pts.pop((I, pr, J))
                r = J - 4 * I
                w0 = 128 * r if r >= 1 else 0
                first, last = (J == 0), (J == 4 * I + 3)
                if first:
                    yts[pr] = (
                        psp.tile([D + 1, 512], f32, tag="ytA", bufs=1,
                                 name=f"ytA{pr}_{I}"),
                        psp.tile([D + 1, 512], f32, tag="ytB", bufs=1,
                                 name=f"ytB{pr}_{I}"),
                    )
                ytA, ytB = yts[pr]
                nc.tensor.matmul(
                    ytA[:, w0:512], v1_sb[:, 2 * pr, J, :], pt[:, w0:512],
                    start=first, stop=last, skip_group_check=True,
                )
                nc.tensor.matmul(
                    ytB[:, w0:512], v1_sb[:, 2 * pr + 1, J, :],
                    pt[:, 512 + w0 : 1024],
                    start=first, stop=last, skip_group_check=True,
                )

            def out_stage(I, pr):
                ytA, ytB = yts.pop(pr)
                tmpA = op.tile([D + 1, 512], f32, tag="tmpA", bufs=2,
                               name=f"tmpA{pr}_{I}")
                tmpB = op.tile([D + 1, 512], f32, tag="tmpB", bufs=2,
                               name=f"tmpB{pr}_{I}")
                nc.vector.tensor_copy(tmpA, ytA)   # frees head-A psum bank
                nc.vector.tensor_copy(tmpB, ytB)   # frees head-B psum bank
                rec = op.tile([1, 1024], f32, tag="rec", bufs=2,
                              name=f"rec{pr}_{I}")
                nc.vector.reciprocal(rec[:, 0:512], tmpA[D : D + 1, :])
                nc.vector.reciprocal(rec[:, 512:1024], tmpB[D : D + 1, :])
                rbc = op.tile([64, 1024], f32, tag="rbc", bufs=2,
                              name=f"rbc{pr}_{I}")
                nc.gpsimd.partition_broadcast(rbc, rec)
                ystage = op.tile([128, 512], f32, tag="ystage", bufs=2,
                                 name=f"ys{pr}_{I}")
                nc.vector.tensor_mul(ystage[0:64, :], tmpA[0:D, :],
                                     rbc[:, 0:512])
                nc.vector.tensor_mul(ystage[64:128, :], tmpB[0:D, :],
                                     rbc[:, 512:1024])
                nc.sync.dma_start(
                    out=yT[pr * 128 : (pr + 1) * 128, I * 512 : (I + 1) * 512],
                    in_=ystage)

            # ---- schedule: one global item stream with spread group weave ----
            # Each unit (I, pr) = all j-tiles of one head-pair/query-block.
            # QKV groups are pre-assigned to the unit ONE consumer earlier and
            # interleaved between its PV calls at an even stride, so the PE
            # stream always has the next unit's inputs ready and ScalarE's
            # exp queue never drains at unit/block boundaries.
            units = [(I, p) for I in range(TB) for p in range(PAIRS)]
            uidx = {u: i for i, u in enumerate(units)}
            items = [(I, p, J) for (I, p) in units for J in range(4 * (I + 1))]

            weave = {u: [] for u in range(len(units))}
            # q/k groups of pair p>=1 -> woven into the previous pair's unit
            for tb in range(TB):
                for p in range(1, PAIRS):
                    weave[uidx[(tb, p)] - 1] += [
                        lambda tb=tb, p=p: g_qk(tb, 2 * p),
                        lambda tb=tb, p=p: g_qk(tb, 2 * p + 1),
                    ]
            # block tb pair-0 q/k + v groups -> late units of block tb-1
            # (v groups of block tb need x(tb), which arrives mid block tb-1)
            weave[uidx[(0, 2)]] += [
                lambda: g_qk(1, 0), lambda: g_qk(1, 1), lambda: g_v(1, 0)]
            weave[uidx[(0, 3)]] += [
                lambda: g_v(1, 1), lambda: g_v(1, 2), lambda: g_v(1, 3)]
            for tb in range(2, TB):
                weave[uidx[(tb - 1, 0)]] += [lambda tb=tb: g_v(tb, 0)]
                weave[uidx[(tb - 1, 1)]] += [lambda tb=tb: g_v(tb, 1)]
                weave[uidx[(tb - 1, 2)]] += [
                    lambda tb=tb: g_qk(tb, 0), lambda tb=tb: g_qk(tb, 1)]
                weave[uidx[(tb - 1, 3)]] += [
                    lambda tb=tb: g_v(tb, 2), lambda tb=tb: g_v(tb, 3)]

            # deadline fallback (correctness): groups not yet woven when a
            # unit's first QK must be emitted are flushed here, in order
            gq_order = []
            for u in range(len(units)):
                for fn in weave[u]:
                    gq_order.append(fn)
            woven = set()

            state = {"emitted": 0}
            seen_units = set()
            unit_first_item = {}
            pos = 0
            for k, (I, p, J) in enumerate(items):
                if J == 0:
                    unit_first_item[uidx[(I, p)]] = k

            def emit_weave(fn):
                if id(fn) not in woven:
                    woven.add(id(fn))
                    fn()

            def flush_for_unit(u):
                # everything assigned to earlier units must be in-stream
                for uu in range(u):
                    for fn in weave[uu]:
                        emit_weave(fn)

            def emit_qk_item(k):
                I, p, J = items[k]
                u = uidx[(I, p)]
                if u not in seen_units:
                    seen_units.add(u)
                    if (I, p) == (1, 0):
                        load_x(2)
                    elif (I, p) == (2, 0):
                        load_x(3)
                    flush_for_unit(u)
                qk_exp(I, p, J)

            g_qk(0, 0)
            g_qk(0, 1)
            for k in range(len(items)):
                while state["emitted"] < min(k + 1, len(items)):
                    emit_qk_item(state["emitted"])
                    state["emitted"] += 1
                I, p, J = items[k]
                u = uidx[(I, p)]
                if k == 0:  # block-0 v groups: after first QKs, before first PV
                    for kk in range(1 + LOOKAHEAD):
                        emit_qk_item(state["emitted"])
                        state["emitted"] += 1
                    for k4 in range(4):
                        g_v(0, k4)
                pv(I, p, J)
                if J == 4 * I + 3:
                    out_stage(I, p)
                while state["emitted"] < min(k + 2 + LOOKAHEAD, len(items)):
                    emit_qk_item(state["emitted"])
                    state["emitted"] += 1
                # spread this unit's weave list across its items, finishing
                # LOOKAHEAD items before the unit ends
                wl = weave[u]
                if wl:
                    n_items = 4 * (I + 1)
                    slots = max(1, n_items - LOOKAHEAD)
                    j_in = k - unit_first_item[u]
                    done = min(len(wl), (j_in + 1) * len(wl) // slots
                               if slots > 1 else len(wl))
                    for fn in wl[:done]:
                        emit_weave(fn)
            for fn in gq_order:
                emit_weave(fn)
    nc.compile()
    return nc


def _mk_ap(src_ap, offset, dims):
    """Raw strided view of a DRAM tensor (strides/offset in elements)."""
    import concourse.bass as bass

    return bass.AP(tensor=src_ap.tensor, offset=offset, ap=dims)


def _bcast_ap(src_ap, nparts):
    """Partition-broadcast view of a [1, N] DRAM AP -> [nparts, N]."""
    import concourse.bass as bass

    return bass.AP(
        tensor=src_ap.tensor,
        offset=src_ap.offset,
        ap=[[0, nparts]] + list(src_ap.ap)[1:],
    )


def get_nc():
    if "nc" not in _cache:
        _cache["nc"] = _build_nc()
    return _cache["nc"]


def shard_inputs(x, w_attn, b_attn):
    """Full inputs -> per-core input maps (host-side slicing/transposition)."""
    import ml_dtypes

    bf16 = ml_dtypes.bfloat16
    x = np.asarray(x, dtype=np.float32)
    w = np.asarray(w_attn, dtype=np.float32)
    bb = np.asarray(b_attn, dtype=np.float32)
    in_maps = []
    for core in range(N_CORES):
        b, hg = core // 2, core % 2
        r0 = hg * CC  # first q row for this head group
        # pair-interleaved q/k weight columns: [q_p0|k_p0|q_p1|k_p1|...]
        blocks = []
        bcols = []
        for p in range(PAIRS):
            blocks.append(w[r0 + 128 * p : r0 + 128 * (p + 1), :].T)
            blocks.append(w[C + r0 + 128 * p : C + r0 + 128 * (p + 1), :].T)
            bcols.append(bb[r0 + 128 * p : r0 + 128 * (p + 1)])
            bcols.append(bb[C + r0 + 128 * p : C + r0 + 128 * (p + 1)])
        w_qk = np.ascontiguousarray(np.concatenate(blocks, axis=1).astype(bf16))
        b_qk = np.ascontiguousarray(np.stack(bcols, axis=1).astype(np.float32))
        w_v = np.ascontiguousarray(
            w[2 * C + r0 : 2 * C + r0 + CC, :].T.astype(bf16))
        b_v = bb[2 * C + r0 : 2 * C + r0 + CC].reshape(1, CC).astype(np.float32)
        in_maps.append(
            {
                "xT": np.ascontiguousarray(x[b].T.astype(bf16)),
                "w_qk": w_qk,
                "w_v": w_v,
                "b_qk": b_qk,
                "b_v": np.ascontiguousarray(b_v),
                "ones_d": np.ones((1, 128), dtype=np.float32),
            }
        )
    return in_maps


def run(in_maps, trace=False, **kw):
    from concourse import bass_utils

    nc = get_nc()
    return bass_utils.run_bass_kernel_spmd(
        nc, in_maps, core_ids=list(range(N_CORES)), trace=trace, **kw
    )


def gather_output(results):
    y = np.empty((B, T, E), dtype=np.float32)
    for core in range(N_CORES):
        b, hg = core // 2, core % 2
        y[b, :, hg * CC : (hg + 1) * CC] = results[core]["yT"].T
    return y


def kernel(x, w_attn, b_attn):
    in_maps = shard_inputs(x, w_attn, b_attn)
    res = run(in_maps, trace=False)
    return gather_output(res.results)
